# revision 56
# baseline (speedup 1.0000x reference)
"""Trainium2 Bass kernel for Conv2DCollapse_w_pillar (pillar scatter -> dense BEV).

Strategy ("one-hot matmul scatter"), data-parallel over batch (1 batch / core):
  - Host: dedup pillar rows per flat cell (last write wins, matching the
    reference), sort by cell, bucket into 256-cell blocks paired 2-per-matmul.
    Features are rounded to a single bf16 plane (harness tolerance is 2e-2
    relative; bf16 rounding contributes ~2e-3) and packed into the exact SBUF
    stationary image per 64-pair chunk: rows 0:K_c hold even blocks (cols
    pair*128+0:64), rows K_c:2K_c odd blocks (cols pair*128+64:128), zero
    quadrants included, so each chunk loads with ONE full-speed contiguous
    DMA.  K_c is the per-chunk max block occupancy across all 8 cores (SPMD
    shares one program); 16-pair output windows are processed in a shared
    occupancy-sorted order so chunks group windows of similar K, minimizing
    padding (output DMAs route each window back to its original span).
  - Device (steady state is DMA-roofline-bound; every engine stays under the
    2.9us/window output-DMA pace): one-hot matrices oh[i, j] = (cell_i == j)
    are built by DVE (10/window) and Pool (6/window) via is_equal; one bf16
    matmul per pair with the block-diagonal stationary scatter+transposes the
    pair into PSUM (128 partitions = 2 blocks x 64 channels; 2-bank PSUM
    tiles x 4 bufs keep matmuls off the drain chain).  ACT and DVE drain
    PSUM to bf16 SBUF (Pool may not touch PSUM), SP issues the dense output
    DMAs (HWDGE issue costs ~600ns of sequencer time, so output DMAs stay
    coarse); the host upcasts bf16 -> f32.  Every output element is written
    exactly once; empty cells get 0 from all-zero one-hot columns.
"""
import sys
sys.path.insert(0, "/opt/trn_rl_repo")
import numpy as np
import ml_dtypes

BF = ml_dtypes.bfloat16
NCORES = 8
C = 64
NX = 512
NY = 512
NXY = NX * NY
BC = 256                 # cells per block
NBLK = NXY // BC         # 1024 blocks per core
NPAIR = NBLK // 2        # 512 pairs per core
CHUNK_PAIRS = 64         # pairs per feature-DMA chunk
NCHUNK = NPAIR // CHUNK_PAIRS
GRP = 4                  # pairs per PSUM group (2 banks; 4 bufs -> matmuls
                         # depend on drains 4 groups back, off the chain)
WIN = 16                 # pairs per output window (one outb / 2 output DMAs)
ACT_COLS = 704           # drain split across a 4-group window: ACT takes
                         # groups 0,2 fully + 704 cols of group 1; DVE takes
                         # 320 of group 1 + group 3 (only ACT/DVE may read
                         # PSUM). 6 of 16 one-hots per window go to Pool.
                         # Keeps every engine under the 2912ns/window DMA pace
NBUF = 5                 # lhs chunk buffers: feature DMA issues 2 chunks ahead
                         # of compute, and the buffer it overwrites went idle
                         # 2 chunks ago, so the issue's embedded wait is stale

_cache = {}


def _build_nc(Ks, wperm):
    import concourse.bass as bass
    import concourse.tile as tile
    from concourse import bacc, mybir
    from contextlib import ExitStack

    dt = mybir.dt
    R = [2 * k for k in Ks]
    offs = np.concatenate([[0], np.cumsum(R)]).tolist()
    W = CHUNK_PAIRS * 128
    nc = bacc.Bacc("TRN2", target_bir_lowering=False, debug=False,
                   num_devices=NCORES)
    feat = nc.dram_tensor("feat", [offs[-1], W], dt.bfloat16,
                          kind="ExternalInput").ap()
    Rmax = max(R)
    cells_d = nc.dram_tensor("cells", [Rmax, NPAIR], dt.float32,
                             kind="ExternalInput").ap()
    iota_d = nc.dram_tensor("iota", [Rmax, BC], dt.bfloat16,
                            kind="ExternalInput").ap()
    out_d = nc.dram_tensor("out", [C, NXY], dt.bfloat16,
                           kind="ExternalOutput").ap()

    with tile.TileContext(nc) as tc, ExitStack() as ctx:
        const = ctx.enter_context(tc.tile_pool(name="const", bufs=1))
        lhsp = ctx.enter_context(tc.tile_pool(name="lhs", bufs=NBUF))
        ohp = ctx.enter_context(tc.tile_pool(name="oh", bufs=32))
        outp = ctx.enter_context(tc.tile_pool(name="outb", bufs=8))
        psp = ctx.enter_context(tc.tile_pool(name="ps", bufs=4, space="PSUM"))

        cells_t = const.tile([Rmax, NPAIR], dt.float32)
        iota_t = const.tile([Rmax, BC], dt.bfloat16)
        # issue from SP FIRST: ACT's queue is stuck behind its act-table load
        # and SP's later feature issues must not beat these small transfers
        # to the DMA FIFO (one-hots need them)
        nc.sync.dma_start(cells_t[:], cells_d[:])
        nc.sync.dma_start(iota_t[:], iota_d[:])

        lhs_t = {}

        def issue_feat(cc):
            t = lhsp.tile([R[cc], W], dt.bfloat16)
            lhs_t[cc] = t
            if cc == 0:
                # quarter the first chunk's transfer so window 0's matmuls
                # start after ~1us of feature data instead of ~4us
                for q in range(4):
                    nc.scalar.dma_start(
                        t[:, q * (W // 4):(q + 1) * (W // 4)],
                        feat[offs[cc]:offs[cc + 1],
                             q * (W // 4):(q + 1) * (W // 4)])
            else:
                # SP issues later chunks so ACT's sequencer stays free for
                # drains during the pipeline-fill phase
                nc.sync.dma_start(t[:], feat[offs[cc]:offs[cc + 1], :])

        for cc in range(min(3, NCHUNK)):
            issue_feat(cc)

        for c in range(NCHUNK):
            if c + 3 < NCHUNK:
                issue_feat(c + 3)
            t = lhs_t.pop(c)
            K2c = R[c]
            p0 = c * CHUNK_PAIRS
            # absorber: consume the feature-DMA sem on PE's clock so the real
            # matmuls only embed their one-hot (DVE) sem waits
            nc.tensor.ldweights(t[0:K2c, 0:128])
            gpw = WIN // GRP
            for g in range(CHUNK_PAIRS // GRP):
                if g % gpw == 0:
                    outb = outp.tile([128, WIN * BC], dt.bfloat16)
                ps_t = psp.tile([128, GRP * BC], dt.float32)
                # Pool takes extra one-hots in the first chunks (it is idle
                # while the window pipeline fills; DMA paces slower there too)
                pool_oh = (2, 4, 5, 7) if c < 2 else (2, 5, 7)
                for i in range(GRP):
                    p = p0 + g * GRP + i
                    oh = ohp.tile([K2c, BC], dt.bfloat16)
                    eng = nc.gpsimd if (g * GRP + i) % 8 in pool_oh else nc.vector
                    eng.tensor_scalar(
                        oh[:], iota_t[0:K2c, :], cells_t[0:K2c, p:p + 1], None,
                        mybir.AluOpType.is_equal)
                    sl = g * GRP + i
                    nc.tensor.matmul(
                        ps_t[:, i * BC:(i + 1) * BC],
                        t[0:K2c, sl * 128:(sl + 1) * 128],
                        oh[:],
                        start=True, stop=True)
                half = (g % gpw) * GRP * BC
                full = GRP * BC
                if g % 4 in (0, 2):
                    nc.scalar.copy(outb[:, half:half + full], ps_t[:])
                elif g % 4 == 1:
                    nc.scalar.copy(outb[:, half:half + ACT_COLS],
                                   ps_t[:, 0:ACT_COLS])
                    nc.vector.tensor_copy(outb[:, half + ACT_COLS:half + full],
                                          ps_t[:, ACT_COLS:full])
                else:
                    nc.vector.tensor_copy(outb[:, half:half + full], ps_t[:])
                if g % gpw == gpw - 1:
                    # windows are processed in occupancy-sorted order (shared
                    # across cores); route each back to its original span
                    slot = (p0 + (g - gpw + 1) * GRP) // WIN
                    base = wperm[slot] * WIN * 2 * BC
                    dst4 = out_d[:, base:base + WIN * 2 * BC].rearrange(
                        "c (p q r) -> c p q r", p=WIN, q=2, r=BC)
                    src_e = outb[0:C, :].rearrange("c (p r) -> c p r", r=BC)
                    src_o = outb[C:128, :].rearrange("c (p r) -> c p r", r=BC)
                    # issue from SP so the multi-sem wait (ACT+Pool drains)
                    # blocks the idle sync sequencer, not ACT's
                    nc.sync.dma_start(dst4[:, :, 0, :], src_e)
                    nc.sync.dma_start(dst4[:, :, 1, :], src_o)
    nc.compile()
    return nc


def _prep_core(pf, cell, Ks, offs, slot_of):
    """pf: (Nb, C) f32 features for this batch (deduped, sorted by cell);
    cell: (Nb,) int cell ids; slot_of[orig_window] -> processing slot."""
    n = len(cell)
    block = cell // BC
    local = (cell % BC).astype(np.float32)
    starts = np.searchsorted(block, np.arange(NBLK))
    k = np.arange(n) - starts[block]
    opair = block // 2
    parity = block % 2
    # remap pairs into occupancy-sorted window slots
    pair = slot_of[opair // WIN] * WIN + opair % WIN
    chunk = pair // CHUNK_PAIRS
    Kc = Ks[chunk]
    assert np.all(k < Kc)

    hi = pf.astype(BF)
    W = CHUNK_PAIRS * 128
    feat = np.zeros((offs[-1], W), dtype=BF)
    row = offs[chunk] + parity * Kc + k
    colb = (pair % CHUNK_PAIRS) * 128 + parity * C
    feat[row[:, None], colb[:, None] + np.arange(C)] = hi

    Rmax = 2 * int(Ks.max())
    cells = np.full((Rmax, NPAIR), -1.0, np.float32)
    cells[parity * Kc + k, pair] = local
    iota = np.broadcast_to(
        np.arange(BC, dtype=np.float32), (Rmax, BC)).astype(BF).copy()
    return {"feat": feat, "cells": cells, "iota": iota}


def kernel(pillar_features, coords, batch_size, nx, ny, num_bev_features,
           **_ignored):
    from concourse import bass_utils

    pf = np.ascontiguousarray(np.asarray(pillar_features, dtype=np.float32))
    co = np.asarray(coords).astype(np.int64)
    B = int(batch_size)
    nx_i, ny_i, C_i = int(nx), int(ny), int(num_bev_features)
    assert (B, nx_i, ny_i, C_i) == (NCORES, NX, NY, C), "hardcoded shape mismatch"

    key = co[:, 0] * NXY + co[:, 1] + co[:, 2] * NX + co[:, 3]
    # dedup, last occurrence wins (matches reference .at[].set semantics)
    n = len(key)
    u, first_rev = np.unique(key[::-1], return_index=True)
    src = n - 1 - first_rev           # original row index that survives
    # u is sorted by (batch, cell)
    batch = (u // NXY).astype(np.int64)
    cell = (u % NXY).astype(np.int64)
    bstart = np.searchsorted(batch, np.arange(NCORES + 1))

    # per-chunk K = max 256-cell-block occupancy across all cores (SPMD: one
    # program shared by the 8 cores).  16-pair windows are sorted by that
    # cross-core occupancy (one shared order) so chunks hold windows of
    # similar K, minimizing padding; output DMAs route each window back to
    # its original span
    po = np.zeros((NCORES, NPAIR), np.int64)
    for b in range(NCORES):
        cb = cell[bstart[b]:bstart[b + 1]]
        occ = np.bincount(cb // BC, minlength=NBLK)
        po[b] = np.maximum(occ[0::2], occ[1::2])
    ccmax = po.max(axis=0)
    wmax = ccmax.reshape(NPAIR // WIN, WIN).max(axis=1)
    wperm = np.argsort(-wmax, kind="stable")      # slot -> original window
    slot_of = np.empty_like(wperm)
    slot_of[wperm] = np.arange(len(wperm))        # original window -> slot
    wpc = CHUNK_PAIRS // WIN                      # windows per chunk
    Ks = wmax[wperm].reshape(NCHUNK, wpc).max(axis=1)
    Ks = tuple(int(max(4, k)) for k in Ks)
    assert max(Ks) <= 64, f"block occupancy {max(Ks)} too high for pair kernel"
    offs = np.concatenate([[0], np.cumsum([2 * k for k in Ks])])

    key_ = (Ks, tuple(int(w) for w in wperm))
    if key_ not in _cache:
        _cache[key_] = _build_nc(Ks, tuple(int(w) for w in wperm))
    nc = _cache[key_]

    in_maps = []
    for b in range(NCORES):
        lo_i, hi_i = bstart[b], bstart[b + 1]
        in_maps.append(_prep_core(pf[src[lo_i:hi_i]], cell[lo_i:hi_i],
                                  np.asarray(Ks), offs, slot_of))

    import os
    trace = bool(os.environ.get("BASS_TRACE"))
    res = bass_utils.run_bass_kernel_spmd(
        nc, in_maps, core_ids=list(range(NCORES)), trace=trace)
    kernel._last_results = res

    out = np.empty((NCORES, C, NY, NX), dtype=np.float32)
    for b in range(NCORES):
        out[b] = res.results[b]["out"].astype(np.float32).reshape(C, NY, NX)
    return out


# revision 59
# speedup vs baseline: 1.0020x; 1.0020x over previous
"""Trainium2 Bass kernel for Conv2DCollapse_w_pillar (pillar scatter -> dense BEV).

Strategy ("one-hot matmul scatter"), data-parallel over batch (1 batch / core):
  - Host: dedup pillar rows per flat cell (last write wins, matching the
    reference), sort by cell, bucket into 256-cell blocks paired 2-per-matmul.
    Features are rounded to a single bf16 plane (harness tolerance is 2e-2
    relative; bf16 rounding contributes ~2e-3) and packed into the exact SBUF
    stationary image per 64-pair chunk: rows 0:K_c hold even blocks (cols
    pair*128+0:64), rows K_c:2K_c odd blocks (cols pair*128+64:128), zero
    quadrants included, so each chunk loads with ONE full-speed contiguous
    DMA.  K_c is the per-chunk max block occupancy across all 8 cores (SPMD
    shares one program); 16-pair output windows are processed in a shared
    occupancy-sorted order so chunks group windows of similar K, minimizing
    padding (output DMAs route each window back to its original span).
  - Device (steady state is DMA-roofline-bound; every engine stays under the
    2.9us/window output-DMA pace): one-hot matrices oh[i, j] = (cell_i == j)
    are built by DVE (10/window) and Pool (6/window) via is_equal; one bf16
    matmul per pair with the block-diagonal stationary scatter+transposes the
    pair into PSUM (128 partitions = 2 blocks x 64 channels; 2-bank PSUM
    tiles x 4 bufs keep matmuls off the drain chain).  ACT and DVE drain
    PSUM to bf16 SBUF (Pool may not touch PSUM), SP issues the dense output
    DMAs (HWDGE issue costs ~600ns of sequencer time, so output DMAs stay
    coarse); the host upcasts bf16 -> f32.  Every output element is written
    exactly once; empty cells get 0 from all-zero one-hot columns.
"""
import sys
sys.path.insert(0, "/opt/trn_rl_repo")
import numpy as np
import ml_dtypes

BF = ml_dtypes.bfloat16
NCORES = 8
C = 64
NX = 512
NY = 512
NXY = NX * NY
BC = 256                 # cells per block
NBLK = NXY // BC         # 1024 blocks per core
NPAIR = NBLK // 2        # 512 pairs per core
CHUNK_PAIRS = 64         # pairs per feature-DMA chunk
NCHUNK = NPAIR // CHUNK_PAIRS
GRP = 4                  # pairs per PSUM group (2 banks; 4 bufs -> matmuls
                         # depend on drains 4 groups back, off the chain)
WIN = 16                 # pairs per output window (one outb / 2 output DMAs)
ACT_COLS = 704           # drain split across a 4-group window: ACT takes
                         # groups 0,2 fully + 704 cols of group 1; DVE takes
                         # 320 of group 1 + group 3 (only ACT/DVE may read
                         # PSUM). 6 of 16 one-hots per window go to Pool.
                         # Keeps every engine under the 2912ns/window DMA pace
NBUF = 5                 # lhs chunk buffers: feature DMA issues 2 chunks ahead
                         # of compute, and the buffer it overwrites went idle
                         # 2 chunks ago, so the issue's embedded wait is stale

_cache = {}


def _build_nc(Ks, wperm):
    import concourse.bass as bass
    import concourse.tile as tile
    from concourse import bacc, mybir
    from contextlib import ExitStack

    dt = mybir.dt
    R = [2 * k for k in Ks]
    offs = np.concatenate([[0], np.cumsum(R)]).tolist()
    W = CHUNK_PAIRS * 128
    nc = bacc.Bacc("TRN2", target_bir_lowering=False, debug=False,
                   num_devices=NCORES)
    feat = nc.dram_tensor("feat", [offs[-1], W], dt.bfloat16,
                          kind="ExternalInput").ap()
    Rmax = max(R)
    cells_d = nc.dram_tensor("cells", [Rmax, NPAIR], dt.float32,
                             kind="ExternalInput").ap()
    out_d = nc.dram_tensor("out", [C, NXY], dt.bfloat16,
                           kind="ExternalOutput").ap()

    with tile.TileContext(nc) as tc, ExitStack() as ctx:
        const = ctx.enter_context(tc.tile_pool(name="const", bufs=1))
        lhsp = ctx.enter_context(tc.tile_pool(name="lhs", bufs=NBUF))
        ohp = ctx.enter_context(tc.tile_pool(name="oh", bufs=32))
        outp = ctx.enter_context(tc.tile_pool(name="outb", bufs=8))
        psp = ctx.enter_context(tc.tile_pool(name="ps", bufs=4, space="PSUM"))

        cells_t = const.tile([Rmax, NPAIR], dt.float32)
        iota_t = const.tile([Rmax, BC], dt.bfloat16)
        # issue from SP FIRST: ACT's queue is stuck behind its act-table load
        # and SP's later feature issues must not beat this small transfer
        # to the DMA FIFO (one-hots need it)
        nc.sync.dma_start(cells_t[:], cells_d[:])
        # build the 0..255 row pattern on Pool (exact in bf16 up to 256):
        # no DMA, ready before cells lands
        nc.gpsimd.iota(iota_t[:], [[1, BC]], base=0, channel_multiplier=0,
                       allow_small_or_imprecise_dtypes=True)

        lhs_t = {}

        def issue_feat(cc):
            t = lhsp.tile([R[cc], W], dt.bfloat16)
            lhs_t[cc] = t
            if cc == 0:
                # quarter the first chunk's transfer so window 0's matmuls
                # start after ~1us of feature data instead of ~4us
                for q in range(4):
                    nc.scalar.dma_start(
                        t[:, q * (W // 4):(q + 1) * (W // 4)],
                        feat[offs[cc]:offs[cc + 1],
                             q * (W // 4):(q + 1) * (W // 4)])
            else:
                # SP issues later chunks so ACT's sequencer stays free for
                # drains during the pipeline-fill phase
                nc.sync.dma_start(t[:], feat[offs[cc]:offs[cc + 1], :])

        for cc in range(min(3, NCHUNK)):
            issue_feat(cc)

        for c in range(NCHUNK):
            if c + 3 < NCHUNK:
                issue_feat(c + 3)
            t = lhs_t.pop(c)
            K2c = R[c]
            p0 = c * CHUNK_PAIRS
            # absorber: consume the feature-DMA sem on PE's clock so the real
            # matmuls only embed their one-hot (DVE) sem waits
            nc.tensor.ldweights(t[0:K2c, 0:128])
            gpw = WIN // GRP
            for g in range(CHUNK_PAIRS // GRP):
                if g % gpw == 0:
                    outb = outp.tile([128, WIN * BC], dt.bfloat16)
                ps_t = psp.tile([128, GRP * BC], dt.float32)
                # Pool takes extra one-hots in the first chunks (it is idle
                # while the window pipeline fills; DMA paces slower there too)
                pool_oh = (2, 4, 5, 7) if c < 2 else (2, 5, 7)
                for i in range(GRP):
                    p = p0 + g * GRP + i
                    oh = ohp.tile([K2c, BC], dt.bfloat16)
                    eng = nc.gpsimd if (g * GRP + i) % 8 in pool_oh else nc.vector
                    eng.tensor_scalar(
                        oh[:], iota_t[0:K2c, :], cells_t[0:K2c, p:p + 1], None,
                        mybir.AluOpType.is_equal)
                    sl = g * GRP + i
                    nc.tensor.matmul(
                        ps_t[:, i * BC:(i + 1) * BC],
                        t[0:K2c, sl * 128:(sl + 1) * 128],
                        oh[:],
                        start=True, stop=True)
                half = (g % gpw) * GRP * BC
                full = GRP * BC
                if g % 4 in (0, 2):
                    nc.scalar.copy(outb[:, half:half + full], ps_t[:])
                elif g % 4 == 1:
                    nc.scalar.copy(outb[:, half:half + ACT_COLS],
                                   ps_t[:, 0:ACT_COLS])
                    nc.vector.tensor_copy(outb[:, half + ACT_COLS:half + full],
                                          ps_t[:, ACT_COLS:full])
                else:
                    nc.vector.tensor_copy(outb[:, half:half + full], ps_t[:])
                if g % gpw == gpw - 1:
                    # windows are processed in occupancy-sorted order (shared
                    # across cores); route each back to its original span
                    slot = (p0 + (g - gpw + 1) * GRP) // WIN
                    base = wperm[slot] * WIN * 2 * BC
                    dst4 = out_d[:, base:base + WIN * 2 * BC].rearrange(
                        "c (p q r) -> c p q r", p=WIN, q=2, r=BC)
                    src_e = outb[0:C, :].rearrange("c (p r) -> c p r", r=BC)
                    src_o = outb[C:128, :].rearrange("c (p r) -> c p r", r=BC)
                    # issue from SP so the multi-sem wait (ACT+Pool drains)
                    # blocks the idle sync sequencer, not ACT's
                    nc.sync.dma_start(dst4[:, :, 0, :], src_e)
                    nc.sync.dma_start(dst4[:, :, 1, :], src_o)
    nc.compile()
    return nc


def _prep_core(pf, cell, Ks, offs, slot_of):
    """pf: (Nb, C) f32 features for this batch (deduped, sorted by cell);
    cell: (Nb,) int cell ids; slot_of[orig_window] -> processing slot."""
    n = len(cell)
    block = cell // BC
    local = (cell % BC).astype(np.float32)
    starts = np.searchsorted(block, np.arange(NBLK))
    k = np.arange(n) - starts[block]
    opair = block // 2
    parity = block % 2
    # remap pairs into occupancy-sorted window slots
    pair = slot_of[opair // WIN] * WIN + opair % WIN
    chunk = pair // CHUNK_PAIRS
    Kc = Ks[chunk]
    assert np.all(k < Kc)

    hi = pf.astype(BF)
    W = CHUNK_PAIRS * 128
    feat = np.zeros((offs[-1], W), dtype=BF)
    row = offs[chunk] + parity * Kc + k
    colb = (pair % CHUNK_PAIRS) * 128 + parity * C
    feat[row[:, None], colb[:, None] + np.arange(C)] = hi

    Rmax = 2 * int(Ks.max())
    cells = np.full((Rmax, NPAIR), -1.0, np.float32)
    cells[parity * Kc + k, pair] = local
    return {"feat": feat, "cells": cells}


def kernel(pillar_features, coords, batch_size, nx, ny, num_bev_features,
           **_ignored):
    from concourse import bass_utils

    pf = np.ascontiguousarray(np.asarray(pillar_features, dtype=np.float32))
    co = np.asarray(coords).astype(np.int64)
    B = int(batch_size)
    nx_i, ny_i, C_i = int(nx), int(ny), int(num_bev_features)
    assert (B, nx_i, ny_i, C_i) == (NCORES, NX, NY, C), "hardcoded shape mismatch"

    key = co[:, 0] * NXY + co[:, 1] + co[:, 2] * NX + co[:, 3]
    # dedup, last occurrence wins (matches reference .at[].set semantics)
    n = len(key)
    u, first_rev = np.unique(key[::-1], return_index=True)
    src = n - 1 - first_rev           # original row index that survives
    # u is sorted by (batch, cell)
    batch = (u // NXY).astype(np.int64)
    cell = (u % NXY).astype(np.int64)
    bstart = np.searchsorted(batch, np.arange(NCORES + 1))

    # per-chunk K = max 256-cell-block occupancy across all cores (SPMD: one
    # program shared by the 8 cores).  16-pair windows are sorted by that
    # cross-core occupancy (one shared order) so chunks hold windows of
    # similar K, minimizing padding; output DMAs route each window back to
    # its original span
    po = np.zeros((NCORES, NPAIR), np.int64)
    for b in range(NCORES):
        cb = cell[bstart[b]:bstart[b + 1]]
        occ = np.bincount(cb // BC, minlength=NBLK)
        po[b] = np.maximum(occ[0::2], occ[1::2])
    ccmax = po.max(axis=0)
    wmax = ccmax.reshape(NPAIR // WIN, WIN).max(axis=1)
    wperm = np.argsort(-wmax, kind="stable")      # slot -> original window
    slot_of = np.empty_like(wperm)
    slot_of[wperm] = np.arange(len(wperm))        # original window -> slot
    wpc = CHUNK_PAIRS // WIN                      # windows per chunk
    Ks = wmax[wperm].reshape(NCHUNK, wpc).max(axis=1)
    Ks = tuple(int(max(4, k)) for k in Ks)
    assert max(Ks) <= 64, f"block occupancy {max(Ks)} too high for pair kernel"
    offs = np.concatenate([[0], np.cumsum([2 * k for k in Ks])])

    key_ = (Ks, tuple(int(w) for w in wperm))
    if key_ not in _cache:
        _cache[key_] = _build_nc(Ks, tuple(int(w) for w in wperm))
    nc = _cache[key_]

    in_maps = []
    for b in range(NCORES):
        lo_i, hi_i = bstart[b], bstart[b + 1]
        in_maps.append(_prep_core(pf[src[lo_i:hi_i]], cell[lo_i:hi_i],
                                  np.asarray(Ks), offs, slot_of))

    import os
    trace = bool(os.environ.get("BASS_TRACE"))
    res = bass_utils.run_bass_kernel_spmd(
        nc, in_maps, core_ids=list(range(NCORES)), trace=trace)
    kernel._last_results = res

    out = np.empty((NCORES, C, NY, NX), dtype=np.float32)
    for b in range(NCORES):
        out[b] = res.results[b]["out"].astype(np.float32).reshape(C, NY, NX)
    return out
